# revision 21
# baseline (speedup 1.0000x reference)
"""CWTConvNet Trainium2 kernel — flipped (weight-Toeplitz) formulation.

The reference reduces exactly (see earlier baseline) to

    out72[f, s, l] = sum_{j=0}^{351} w2[f, j] * xe[s, j + l],  l in [0, 72)

with w2 = w_real[:, 0, 209:561] and xe = [71 zeros, x[s, 0:352], pad], then an
index-repeat expansion 72 -> 224 (IMG_SELECT) on the host.

Instead of im2col-ing the DATA (2.43 MB of HBM reads per core), this kernel
im2cols the WEIGHTS, which are shared by all 48 signals on a core:

    l = 9*lb + l',  lb in [0,8), l' in [0,9)
    OUT[(s,lb), (f,l')] = sum_c sum_p  xe[s, 128c + p + 9*lb] * w2[f, 128c + p - l']

Per pass of 16 signals the stationary operand is the (tiny, host-built) data
tile stat[c][p, (s,lb)] = xe[s, 128c+p+9lb] (128x128 — full PE array), and
the moving rhs is the weight-Toeplitz wt[c][p, (f,l')] = w2[f, 128c+p-l'],
which is signal-independent. Filter support limits chunk 1 to filters
48..111 and chunk 2 to filters 104..111 with rows p < 104, so the weight
bank is only 1008+576+72 columns. Per-core HBM traffic drops from ~4.1 MB
(baseline) to ~1.5 MB: ~720 KB of loads + 774 KB of bf16 stores.

Scheduling structure:
  - Load 1 carries everything pass 0 needs, so pass-0 matmuls, drains and
    store start ~1 us before the pass-1/2 stationaries (loads 2/3) finish.
  - The PSUM filter->bank split is PERMUTED (bank A = filters 0..47 +
    104..111, bank B = 48..103) so bank B's accumulation ends one matmul
    early and its drain overlaps the tiny chunk-2 matmul.
  - A burst of zero matmuls into a scratch bank keeps the PE busy from
    preamble end, opening the HAM clock gate before the real matmuls.
  - The final pass stores per-bank on both HWDGE rings to shorten the tail.
"""

import numpy as np

import concourse.bacc as bacc
import concourse.bass as bass
import concourse.mybir as mybir
import concourse.tile as tile
from concourse.bass_utils import run_bass_kernel_spmd

# Problem constants (hardcoded; kernel.py must be self-contained).
B, C, L = 32, 12, 2048
F, K = 112, 561
KOFF = 209                 # first needed tap; w2 = w_real[:, 0, 209:561]
J = 352                    # taps per filter window
NCORES = 8
BPC = B // NCORES          # batches per core
S = BPC * C                # signals per core (48)
NL = 72                    # conv output positions actually used
NI = 224                   # expanded output length

NT = 3                     # passes per core (16 signals each)
SG = 16                    # signals per pass
NLB, NLP = 8, 9            # l = NLP*lb + l'; SG*NLB = 128 partitions exactly
F1LO, F2LO = 48, 104       # first filter with support in chunks 1 / 2
C2ROWS = 104               # chunk-2 rows p >= 104 are all-zero weights
XE_LEN = 448               # 71 zeros + 352 signal + tail (max index 446)
XE_ZLEAD = 71
NDUMMY = 4                 # PE warm-up matmuls while loads stream

# PSUM bank permutation: bank A = filters 0..47 + 104..111, bank B = 48..103.
PERM = np.r_[np.arange(0, 48), np.arange(104, 112), np.arange(48, 104)]
NBK = 56 * NLP             # 504 columns per bank

# Load split (5 DMAs, ordered by first use):
#   L1 = stat00 | wt0a           (632 cols)  -> first matmul ~1.4us earlier
#   L2 = wt0b | stat01 | stat02  (760 cols)
#   L3 = wt1a | wt1b | wt2       (648 cols)  -> completes pass 0
#   L4 = stats pass 1            (384 cols)
#   L5 = stats pass 2            (384 cols)
L1_COLS = 128 + 504
L2_COLS = 504 + 128 + 128
L3_COLS = 72 + 504 + 72

SEL = np.linspace(0, 71, NI, dtype=np.int64)

_CACHE = {}


def _build_nc():
    f32 = mybir.dt.float32
    bf16 = mybir.dt.bfloat16
    nc = bacc.Bacc("TRN2", target_bir_lowering=False, debug=False)

    pack1_d = nc.declare_dram_parameter("pack1", [128, L1_COLS], bf16, isOutput=False)
    pack1b_d = nc.declare_dram_parameter("pack1b", [128, L2_COLS], bf16, isOutput=False)
    pack1c_d = nc.declare_dram_parameter("pack1c", [128, L3_COLS], bf16, isOutput=False)
    pack2_d = nc.declare_dram_parameter("pack2", [128, 384], bf16, isOutput=False)
    pack3_d = nc.declare_dram_parameter("pack3", [128, 384], bf16, isOutput=False)
    y_d = nc.declare_dram_parameter("y", [NT, 128, 2 * NBK], bf16, isOutput=True)

    with tile.TileContext(nc) as tc:
        with (
            tc.tile_pool(name="sbuf", bufs=1) as pool,
            tc.tile_pool(name="psum", bufs=1, space="PSUM") as psum_pool,
        ):
            # Warm-up scratch memset first so the vector engine clears it
            # the moment its preamble ends (the dummy matmuls chase it).
            scratch = pool.tile([128, 504], bf16, tag="scr", name="scr")
            nc.vector.memset(scratch[:], 0)

            # Loads next so the sync engine issues them back to back.
            big1 = pool.tile([128, L1_COLS], bf16, tag="big1", name="big1")
            big1b = pool.tile([128, L2_COLS], bf16, tag="big1b", name="big1b")
            big1c = pool.tile([128, L3_COLS], bf16, tag="big1c", name="big1c")
            big2 = pool.tile([128, 384], bf16, tag="big2", name="big2")
            big3 = pool.tile([128, 384], bf16, tag="big3", name="big3")
            nc.sync.dma_start(out=big1[:], in_=pack1_d.ap())
            nc.sync.dma_start(out=big1b[:], in_=pack1b_d.ap())
            nc.sync.dma_start(out=big1c[:], in_=pack1c_d.ap())
            nc.sync.dma_start(out=big2[:], in_=pack2_d.ap())
            nc.sync.dma_start(out=big3[:], in_=pack3_d.ap())

            # PE HAM warm-up: back-to-back zero matmuls into a scratch bank
            # while the loads stream — continuous PE activity from preamble
            # end opens the clock gate before the real matmuls start.
            ps_dummy = psum_pool.tile([128, 504], f32, tag="psD", name="psD")
            for _ in range(NDUMMY):
                nc.tensor.matmul(
                    ps_dummy[:, :], scratch[:, 0:128], scratch[:, :],
                    start=True, stop=True,
                )

            wt0a = big1[:, 128 : 128 + 504]
            wt0b = big1b[:, 0:504]
            st01 = big1b[:, 504:632]
            st02 = big1b[:, 632:760]
            wt1a = big1c[:, 0:72]
            wt1b = big1c[:, 72:576]
            wt2 = big1c[:, 576:648]
            stat = {
                (0, 0): big1[:, 0:128], (0, 1): st01, (0, 2): st02,
                (1, 0): big2[:, 0:128], (1, 1): big2[:, 128:256], (1, 2): big2[:, 256:384],
                (2, 0): big3[:, 0:128], (2, 1): big3[:, 128:256], (2, 2): big3[:, 256:384],
            }

            for t in range(NT):
                psA = psum_pool.tile([128, NBK], f32, tag=f"psA{t}", name=f"psA{t}")
                psB = psum_pool.tile([128, NBK], f32, tag=f"psB{t}", name=f"psB{t}")
                stA, stB, stC = stat[(t, 0)], stat[(t, 1)], stat[(t, 2)]
                # chunk 0 (all filters), stationary stA
                nc.tensor.matmul(psA[:, :], stA, wt0a, start=True, stop=False)
                nc.tensor.matmul(psB[:, :], stA, wt0b, start=True, stop=False)
                # chunk 1: bank-A slice is filters 104..111, bank B 48..103
                nc.tensor.matmul(psA[:, 432:504], stB, wt1a, start=False, stop=False)
                nc.tensor.matmul(psB[:, :], stB, wt1b, start=False, stop=True)
                # chunk 2 (filters 104..111, rows < 104) finishes bank A
                nc.tensor.matmul(
                    psA[:, 432:504], stC[0:C2ROWS, :], wt2[0:C2ROWS, :],
                    start=False, stop=True,
                )
                # Bank B completes first: drain it on the vector engine
                # under the chunk-2 matmul; bank A drains on scalar right
                # after. Final pass stores per-bank on both HWDGE rings.
                ot = pool.tile([128, 2 * NBK], bf16, tag=f"o{t}", name=f"o{t}")
                nc.vector.tensor_copy(out=ot[:, NBK : 2 * NBK], in_=psB[:, :])
                nc.scalar.copy(ot[:, 0:NBK], psA[:, :])
                if t < NT - 1:
                    nc.sync.dma_start(out=y_d.ap()[t], in_=ot[:])
                else:
                    nc.sync.dma_start(
                        out=y_d.ap()[t][:, NBK : 2 * NBK], in_=ot[:, NBK : 2 * NBK]
                    )
                    nc.scalar.dma_start(
                        out=y_d.ap()[t][:, 0:NBK], in_=ot[:, 0:NBK]
                    )

    nc.compile()
    return nc


def _get_nc():
    if "nc" not in _CACHE:
        _CACHE["nc"] = _build_nc()
    return _CACHE["nc"]


def _build_wt(w2):
    """Weight-Toeplitz chunks, permuted into the bank order.

    wt_c[p, cols] with cols enumerating (filter, l') pairs; filter order is
    PERM for chunk 0, [104..111, 48..103] for chunk 1, [104..111] for 2.
    """
    def toep(c, fsel, rows):
        p = np.arange(128)[:, None, None]
        f = np.asarray(fsel)[None, :, None]
        lp = np.arange(NLP)[None, None, :]
        j = 128 * c + p - lp
        val = np.where((j >= 0) & (j < J), w2[f, np.clip(j, 0, J - 1)], 0.0)
        val[rows:] = 0.0
        return val.reshape(128, len(fsel) * NLP).astype(np.float32)

    wt0 = toep(0, PERM, 128)                               # [128, 1008]
    wt1 = toep(1, np.r_[np.arange(104, 112), np.arange(48, 104)], 128)
    wt2 = toep(2, np.arange(104, 112), C2ROWS)
    return wt0[:, 0:504], wt0[:, 504:1008], wt1[:, 0:72], wt1[:, 72:576], wt2


def _prepare_in_maps(x, w_real):
    import ml_dtypes

    np_bf16 = np.dtype(ml_dtypes.bfloat16)
    x = np.ascontiguousarray(np.asarray(x), dtype=np.float32)
    w_real = np.asarray(w_real, dtype=np.float32)

    w2 = w_real[:, 0, KOFF:K]                              # [112, 352]
    wt0a, wt0b, wt1a, wt1b, wt2 = _build_wt(w2)

    # Stationary index grid: q[c][p, sl, lb] = 128c + p + 9lb
    p = np.arange(128)[:, None, None]
    lb = np.arange(NLB)[None, None, :]
    qs = [128 * c + p + NLP * lb for c in range(3)]        # each [128, 1, 8]

    in_maps = []
    for m in range(NCORES):
        xc = x[m * BPC : (m + 1) * BPC].reshape(S, L)
        xe = np.zeros((S, XE_LEN), np.float32)
        xe[:, XE_ZLEAD : XE_ZLEAD + J] = xc[:, :J]
        st = {}
        for t in range(NT):
            sig = xe[SG * t : SG * (t + 1)]                # [16, 448]
            for c in range(3):
                v = sig[np.arange(SG)[None, :, None], qs[c]]
                st[(t, c)] = v.reshape(128, SG * NLB)      # col = sl*8 + lb
        pack1 = np.concatenate([st[(0, 0)], wt0a], axis=1)
        pack1b = np.concatenate([wt0b, st[(0, 1)], st[(0, 2)]], axis=1)
        pack1c = np.concatenate([wt1a, wt1b, wt2], axis=1)
        pack2 = np.concatenate([st[(1, c)] for c in range(3)], axis=1)
        pack3 = np.concatenate([st[(2, c)] for c in range(3)], axis=1)
        in_maps.append({
            "pack1": np.ascontiguousarray(pack1).astype(np_bf16),
            "pack1b": np.ascontiguousarray(pack1b).astype(np_bf16),
            "pack1c": np.ascontiguousarray(pack1c).astype(np_bf16),
            "pack2": np.ascontiguousarray(pack2).astype(np_bf16),
            "pack3": np.ascontiguousarray(pack3).astype(np_bf16),
        })
    return in_maps


def _assemble(results):
    # Device output y[t, sl*8+lb, 9*i+l'] = out72[PERM[i], 16t+sl, 9lb+l'].
    ydev = np.stack([np.asarray(r["y"], dtype=np.float32) for r in results])
    yv = ydev.reshape(NCORES, NT, SG, NLB, F, NLP)
    o = yv.transpose(0, 1, 2, 4, 3, 5).reshape(NCORES, S, F, NL)
    out72 = np.empty_like(o)
    out72[:, :, PERM, :] = o                               # undo bank permutation
    y = out72[..., SEL]                                    # [8, S, F, NI]
    return np.ascontiguousarray(y.reshape(B, C, F, NI), dtype=np.float32)


def kernel(x, w_real):
    nc = _get_nc()
    in_maps = _prepare_in_maps(x, w_real)
    res = run_bass_kernel_spmd(nc, in_maps, list(range(NCORES)))
    return _assemble(res.results)


# revision 27
# speedup vs baseline: 1.1249x; 1.1249x over previous
"""CWTConvNet Trainium2 kernel — flipped (weight-Toeplitz) formulation.

The reference reduces exactly (see earlier baseline) to

    out72[f, s, l] = sum_{j=0}^{351} w2[f, j] * xe[s, j + l],  l in [0, 72)

with w2 = w_real[:, 0, 209:561] and xe = [71 zeros, x[s, 0:352], pad], then an
index-repeat expansion 72 -> 224 (IMG_SELECT) on the host.

Instead of im2col-ing the DATA (2.43 MB of HBM reads per core), this kernel
im2cols the WEIGHTS, which are shared by all 48 signals on a core:

    l = 9*lb + l',  lb in [0,8), l' in [0,9)
    OUT[(s,lb), (f,l')] = sum_c sum_p  xe[s, 128c + p + 9*lb] * w2[f, 128c + p - l']

Per pass of 16 signals the stationary operand is the (tiny, host-built) data
tile stat[c][p, (s,lb)] = xe[s, 128c+p+9lb] (128x128 — full PE array), and
the moving rhs is the weight-Toeplitz wt[c][p, (f,l')] = w2[f, 128c+p-l'],
which is signal-independent. Filter support limits chunk 1 to filters
48..111 and chunk 2 to filters 104..111 with rows p < 104, so the weight
bank is only 1008+576+72 columns. Per-core HBM traffic drops from ~4.1 MB
(baseline) to ~1.5 MB: ~720 KB of loads + 774 KB of bf16 stores.

Scheduling structure:
  - Load 1 carries everything pass 0 needs, so pass-0 matmuls, drains and
    store start ~1 us before the pass-1/2 stationaries (loads 2/3) finish.
  - The PSUM filter->bank split is PERMUTED (bank A = filters 0..47 +
    104..111, bank B = 48..103) so bank B's accumulation ends one matmul
    early and its drain overlaps the tiny chunk-2 matmul.
  - A burst of zero matmuls into a scratch bank keeps the PE busy from
    preamble end, opening the HAM clock gate before the real matmuls.
  - The final pass stores per-bank on both HWDGE rings to shorten the tail.
"""

import numpy as np

import concourse.bacc as bacc
import concourse.bass as bass
import concourse.mybir as mybir
import concourse.tile as tile
from concourse.bass_utils import run_bass_kernel_spmd

# Problem constants (hardcoded; kernel.py must be self-contained).
B, C, L = 32, 12, 2048
F, K = 112, 561
KOFF = 209                 # first needed tap; w2 = w_real[:, 0, 209:561]
J = 352                    # taps per filter window
NCORES = 8
BPC = B // NCORES          # batches per core
S = BPC * C                # signals per core (48)
NL = 72                    # conv output positions actually used
NI = 224                   # expanded output length

NT = 3                     # passes per core (16 signals each)
SG = 16                    # signals per pass
NLB, NLP = 8, 9            # l = NLP*lb + l'; SG*NLB = 128 partitions exactly
F1LO, F2LO = 48, 104       # first filter with support in chunks 1 / 2
C2ROWS = 104               # chunk-2 rows p >= 104 are all-zero weights
XE_LEN = 448               # 71 zeros + 352 signal + tail (max index 446)
XE_ZLEAD = 71
NDUMMY = 7                 # PE warm-up matmuls while loads stream

# PSUM bank permutation: bank A = filters 0..47 + 104..111, bank B = 48..103.
PERM = np.r_[np.arange(0, 48), np.arange(104, 112), np.arange(48, 104)]
NBK = 56 * NLP             # 504 columns per bank

# Load split (2 DMAs). Splitting finer does NOT help: the per-DMA
# completion semaphore fires ~0.65us after the data lands on an idle ring
# but ~2us on a busy one, so early sub-loads never unblock earlier (v6
# measured a 3us regression). Only pack1's semaphore gates compute; the
# pass-1/2 stationaries arrive well before the PE reaches them.
#   pack1 = stat00 | wt0a | wt0b | stat01 | stat02 | wt1a | wt1b | wt2
#   pack2 = stats pass 1 | stats pass 2
P1_COLS = 128 + 504 + 504 + 128 + 128 + 72 + 504 + 72   # 2040

SEL = np.linspace(0, 71, NI, dtype=np.int64)

_CACHE = {}


def _build_nc():
    f32 = mybir.dt.float32
    bf16 = mybir.dt.bfloat16
    nc = bacc.Bacc("TRN2", target_bir_lowering=False, debug=False)

    pack1_d = nc.declare_dram_parameter("pack1", [128, P1_COLS], bf16, isOutput=False)
    pack2_d = nc.declare_dram_parameter("pack2", [128, 768], bf16, isOutput=False)
    y_d = nc.declare_dram_parameter("y", [NT, 128, 2 * NBK], bf16, isOutput=True)

    with tile.TileContext(nc) as tc:
        with (
            tc.tile_pool(name="sbuf", bufs=1) as pool,
            tc.tile_pool(name="psum", bufs=1, space="PSUM") as psum_pool,
        ):
            # Warm-up scratch memset first so the vector engine clears it
            # the moment its preamble ends (the dummy matmuls chase it).
            scratch = pool.tile([128, 504], bf16, tag="scr", name="scr")
            nc.vector.memset(scratch[:], 0)

            # Loads next so the sync engine issues them back to back.
            big1 = pool.tile([128, P1_COLS], bf16, tag="big1", name="big1")
            big2 = pool.tile([128, 768], bf16, tag="big2", name="big2")
            nc.sync.dma_start(out=big1[:], in_=pack1_d.ap())
            nc.sync.dma_start(out=big2[:], in_=pack2_d.ap())

            # PE HAM warm-up: back-to-back zero matmuls into a scratch bank
            # while the loads stream — continuous PE activity from preamble
            # end opens the clock gate before the real matmuls start.
            ps_dummy = psum_pool.tile([128, 504], f32, tag="psD", name="psD")
            for _ in range(NDUMMY):
                nc.tensor.matmul(
                    ps_dummy[:, :], scratch[:, 0:128], scratch[:, :],
                    start=True, stop=True,
                )

            o = 128
            wt0a = big1[:, o : o + 504]; o += 504
            wt0b = big1[:, o : o + 504]; o += 504
            st01 = big1[:, o : o + 128]; o += 128
            st02 = big1[:, o : o + 128]; o += 128
            wt1a = big1[:, o : o + 72]; o += 72
            wt1b = big1[:, o : o + 504]; o += 504
            wt2 = big1[:, o : o + 72]
            stat = {
                (0, 0): big1[:, 0:128], (0, 1): st01, (0, 2): st02,
                (1, 0): big2[:, 0:128], (1, 1): big2[:, 128:256], (1, 2): big2[:, 256:384],
                (2, 0): big2[:, 384:512], (2, 1): big2[:, 512:640], (2, 2): big2[:, 640:768],
            }

            for t in range(NT):
                psA = psum_pool.tile([128, NBK], f32, tag=f"psA{t}", name=f"psA{t}")
                psB = psum_pool.tile([128, NBK], f32, tag=f"psB{t}", name=f"psB{t}")
                stA, stB, stC = stat[(t, 0)], stat[(t, 1)], stat[(t, 2)]
                # chunk 0 (all filters), stationary stA
                nc.tensor.matmul(psA[:, :], stA, wt0a, start=True, stop=False)
                nc.tensor.matmul(psB[:, :], stA, wt0b, start=True, stop=False)
                # chunk 1: bank-A slice is filters 104..111, bank B 48..103
                nc.tensor.matmul(psA[:, 432:504], stB, wt1a, start=False, stop=False)
                nc.tensor.matmul(psB[:, :], stB, wt1b, start=False, stop=True)
                # chunk 2 (filters 104..111, rows < 104) finishes bank A
                nc.tensor.matmul(
                    psA[:, 432:504], stC[0:C2ROWS, :], wt2[0:C2ROWS, :],
                    start=False, stop=True,
                )
                # Bank B completes first: drain it on the vector engine
                # under the chunk-2 matmul; bank A drains on scalar right
                # after. Final pass stores per-bank on both HWDGE rings.
                ot = pool.tile([128, 2 * NBK], bf16, tag=f"o{t}", name=f"o{t}")
                nc.vector.tensor_copy(out=ot[:, NBK : 2 * NBK], in_=psB[:, :])
                nc.scalar.copy(ot[:, 0:NBK], psA[:, :])
                if t < NT - 1:
                    nc.sync.dma_start(out=y_d.ap()[t], in_=ot[:])
                else:
                    nc.sync.dma_start(
                        out=y_d.ap()[t][:, NBK : 2 * NBK], in_=ot[:, NBK : 2 * NBK]
                    )
                    nc.scalar.dma_start(
                        out=y_d.ap()[t][:, 0:NBK], in_=ot[:, 0:NBK]
                    )

    nc.compile()
    return nc


def _get_nc():
    if "nc" not in _CACHE:
        _CACHE["nc"] = _build_nc()
    return _CACHE["nc"]


def _build_wt(w2):
    """Weight-Toeplitz chunks, permuted into the bank order.

    wt_c[p, cols] with cols enumerating (filter, l') pairs; filter order is
    PERM for chunk 0, [104..111, 48..103] for chunk 1, [104..111] for 2.
    """
    def toep(c, fsel, rows):
        p = np.arange(128)[:, None, None]
        f = np.asarray(fsel)[None, :, None]
        lp = np.arange(NLP)[None, None, :]
        j = 128 * c + p - lp
        val = np.where((j >= 0) & (j < J), w2[f, np.clip(j, 0, J - 1)], 0.0)
        val[rows:] = 0.0
        return val.reshape(128, len(fsel) * NLP).astype(np.float32)

    wt0 = toep(0, PERM, 128)                               # [128, 1008]
    wt1 = toep(1, np.r_[np.arange(104, 112), np.arange(48, 104)], 128)
    wt2 = toep(2, np.arange(104, 112), C2ROWS)
    return wt0[:, 0:504], wt0[:, 504:1008], wt1[:, 0:72], wt1[:, 72:576], wt2


def _prepare_in_maps(x, w_real):
    import ml_dtypes

    np_bf16 = np.dtype(ml_dtypes.bfloat16)
    x = np.ascontiguousarray(np.asarray(x), dtype=np.float32)
    w_real = np.asarray(w_real, dtype=np.float32)

    w2 = w_real[:, 0, KOFF:K]                              # [112, 352]
    wt0a, wt0b, wt1a, wt1b, wt2 = _build_wt(w2)

    # Stationary index grid: q[c][p, sl, lb] = 128c + p + 9lb
    p = np.arange(128)[:, None, None]
    lb = np.arange(NLB)[None, None, :]
    qs = [128 * c + p + NLP * lb for c in range(3)]        # each [128, 1, 8]

    in_maps = []
    for m in range(NCORES):
        xc = x[m * BPC : (m + 1) * BPC].reshape(S, L)
        xe = np.zeros((S, XE_LEN), np.float32)
        xe[:, XE_ZLEAD : XE_ZLEAD + J] = xc[:, :J]
        st = {}
        for t in range(NT):
            sig = xe[SG * t : SG * (t + 1)]                # [16, 448]
            for c in range(3):
                v = sig[np.arange(SG)[None, :, None], qs[c]]
                st[(t, c)] = v.reshape(128, SG * NLB)      # col = sl*8 + lb
        pack1 = np.concatenate(
            [st[(0, 0)], wt0a, wt0b, st[(0, 1)], st[(0, 2)], wt1a, wt1b, wt2],
            axis=1,
        )
        pack2 = np.concatenate(
            [st[(t, c)] for t in (1, 2) for c in range(3)], axis=1
        )
        in_maps.append({
            "pack1": np.ascontiguousarray(pack1).astype(np_bf16),
            "pack2": np.ascontiguousarray(pack2).astype(np_bf16),
        })
    return in_maps


def _assemble(results):
    # Device output y[t, sl*8+lb, 9*i+l'] = out72[PERM[i], 16t+sl, 9lb+l'].
    ydev = np.stack([np.asarray(r["y"], dtype=np.float32) for r in results])
    yv = ydev.reshape(NCORES, NT, SG, NLB, F, NLP)
    o = yv.transpose(0, 1, 2, 4, 3, 5).reshape(NCORES, S, F, NL)
    out72 = np.empty_like(o)
    out72[:, :, PERM, :] = o                               # undo bank permutation
    y = out72[..., SEL]                                    # [8, S, F, NI]
    return np.ascontiguousarray(y.reshape(B, C, F, NI), dtype=np.float32)


def kernel(x, w_real):
    nc = _get_nc()
    in_maps = _prepare_in_maps(x, w_real)
    res = run_bass_kernel_spmd(nc, in_maps, list(range(NCORES)))
    return _assemble(res.results)
